# revision 52
# baseline (speedup 1.0000x reference)
"""Trainium2 Bass kernel for nn_CrossAttentionBlock (B=2, S=2048, D=1024, H=16, HD=64).

Sharding: 8 cores = 2 batches x 4 head-quads (4 heads each, E=256 channels).
Each core computes q/k/v projections for its quad, RoPE, SDPA, and a partial
output projection [S, D] (fp16); host sums the 4 partials per batch + bo.

Software-pipelined single pass per core. The scalar engine (exp, 128
[128,1024] tiles, ~1.1us each) and the tensor engine (~960 matmuls at
~0.4us issue cost each) are both near-saturated; everything else hides
under them:
  - startup is minimal: k-projection chunk 0, v s-tiles 0..3, q-projection
    of window 0. The rest of k/v projection enters window 0 as
    deadline-scheduled PE filler items, so the exp stream starts ~20us in.
  - 4 SDPA windows (512 q each), 32 units each (16 k-tiles x 2 head-pairs):
    scores pair (row-tiled) -> exp (double-buffered scores PSUM) -> ctx pair
    (col-tiled, PSUM accum). Denominators: exp tiles are pair-summed on DVE
    into fp16 (exact at these magnitudes), halving the ones-matmul count.
  - PSUM->SBUF casts run on GpSimd, rope swaps on the sync queue, output
    stores + normalize broadcast DMAs on GpSimd: DVE keeps only rope
    multiplies, pair-sums, normalize, reciprocal.
  - RoPE: even/odd permutation folded into w_q/w_k rows host-side ->
    rot-half rope out = q*cos + swap_halves(q)*sin_signed.
"""
import os
import sys

sys.path.insert(0, "/opt/trn_rl_repo")

import numpy as np
import ml_dtypes

BF16 = ml_dtypes.bfloat16

B, S, D, H = 2, 2048, 1024, 16
HD = D // H          # 64
DIM = HD // 2        # 32
QUADS = 4            # head groups of 4
E = D // QUADS       # 256 channels per core
ROPE_BASE = 10000.0
N_CORES = 8

KT = D // 128        # 8 k-tiles of the contraction dim
ST = S // 128        # 16 s-tiles
QC = S // 512        # 4 q-chunks (SDPA windows)


def _host_prep(x_q, x_kv, wq, bq, wk, bk, wv, bv, wo):
    """Per-core input maps, every tensor already in its SBUF layout."""
    perm = np.concatenate([np.arange(0, HD, 2), np.arange(1, HD, 2)])  # even|odd
    scale = 1.0 / np.sqrt(HD)

    freqs = np.exp(-np.arange(DIM, dtype=np.float64) * np.log(ROPE_BASE) / DIM)
    ang = np.arange(S, dtype=np.float64)[:, None] * freqs[None, :]     # [S, 32]
    cos = np.cos(ang).T                                                # [32, S]
    sin = np.sin(ang).T
    cos64 = np.concatenate([cos, cos], axis=0)                         # [64, S]
    sin64 = np.concatenate([-sin, sin], axis=0)
    cosT = np.concatenate([cos64, cos64], axis=0).astype(BF16)         # [128, S]
    sinT = np.concatenate([sin64, sin64], axis=0).astype(BF16)

    def x_layout(x):
        # x [S, D] -> xT [D, S] -> sc-major SBUF layout [128, 4*4096]:
        # chunk sc at cols sc*4096, inside: k-tile kt at +kt*512
        xT = x.T.reshape(KT, 128, QC, 512)
        return np.ascontiguousarray(
            xT.transpose(1, 2, 0, 3).reshape(128, KT * S)).astype(BF16)

    def w_layout(w, permute, s):
        # quad rows [256, 1024] (maybe permuted per head, scaled) -> d-major
        # [1024, 256] -> SBUF [128, 8*256] (k-tile kt at cols kt*256)
        blocks = []
        for h in range(4):
            wb = w[h * HD:(h + 1) * HD, :]
            if permute:
                wb = wb[perm, :]
            blocks.append(wb * s)
        wT = np.concatenate(blocks, axis=0).T                          # [1024, 256]
        return np.ascontiguousarray(
            wT.reshape(KT, 128, E).transpose(1, 0, 2).reshape(128, KT * E)
        ).astype(BF16)

    in_maps = []
    for c in range(N_CORES):
        b_ = c // QUADS
        g = c % QUADS
        hs = slice(g * E, (g + 1) * E)
        woT = wo[:, hs].T                                              # [256, 1024]
        wo_dev = np.ascontiguousarray(
            woT.reshape(2, 128, D).transpose(1, 0, 2).reshape(128, 2 * D)
        ).astype(BF16)
        in_maps.append({
            "xq": x_layout(x_q[b_]), "xkv": x_layout(x_kv[b_]),
            "wq": w_layout(wq[hs, :], True, scale),
            "wk": w_layout(wk[hs, :], True, 1.0),
            "wv": w_layout(wv[hs, :], False, 1.0),
            "wo": wo_dev,
            "cosT": np.ascontiguousarray(cosT),
            "sinT": np.ascontiguousarray(sinT),
            "ones_col": np.ones((128, 1), dtype=np.float16),
        })
    return in_maps


# ---------------------------------------------------------------------------
_PROGRAM_CACHE = {}


def _fixed_tile_context(tile_mod, bass_rust_mod, vector_clock_mod):
    """TileContext whose tail drain splits multi-sem waits into single-wait
    NOPs (this walrus rejects >1 sync-wait on one instruction)."""
    SyncInfo = bass_rust_mod.SyncInfo
    ScopedClock = vector_clock_mod.ScopedClock

    class TC(tile_mod.TileContext):
        def _drain_and_barrier(self, tick_clock, wait_clock):
            harvest = self.nc.sync.nop(nofuse=True)
            wait_clock.add_sem_waits(
                harvest.ins, ScopedClock({None: tick_clock.global_clock}))
            si = harvest.ins.sync_info
            waits = list(si.on_wait) if si is not None else []
            if len(waits) > 1:
                harvest.ins.sync_info = SyncInfo(
                    on_wait=[waits[0]], on_update=list(si.on_update))
                for w in waits[1:]:
                    nop = self.nc.sync.nop(nofuse=True)
                    nop.ins.sync_info = SyncInfo(on_wait=[w], on_update=[])
            self.nc.sync.drain()
            self.nc.all_engine_barrier()
            assert self.sems is not None
            popped = self.nc._tile_sem_poison_stack.pop()
            assert popped is self._sem_poison
            self.nc.clear_and_free_semaphores(list(self.sems.allocated().values()))
            self.nc.all_engine_barrier()

    return TC


def _split_multiwait_instructions(nc, mybir, SyncInfo):
    """This walrus build rejects >1 sync-wait per instruction; hoist extra
    waits onto single-wait NOPs inserted just before, on the same engine."""
    ctr = 0
    for blk in nc.m.functions[0].blocks:
        insts = blk.instructions
        i = 0
        while i < len(insts):
            inst = insts[i]
            si = inst.sync_info
            if si is not None and len(si.on_wait) > 1:
                waits = list(si.on_wait)
                inst.sync_info = SyncInfo(on_wait=[waits[-1]],
                                          on_update=list(si.on_update))
                nops = []
                for w in waits[:-1]:
                    nop = mybir.InstNoOp(name=f"waitsplit_{ctr}", ins=[], outs=[])
                    ctr += 1
                    nop.engine = inst.engine
                    nop.sync_info = SyncInfo(on_wait=[w], on_update=[])
                    nops.append(nop)
                insts[i:i] = nops
                i += len(nops)
            i += 1
    return ctr


def build_program(split_waits=True):
    import concourse.bass as bass
    import concourse.mybir as mybir
    import concourse.tile as tile
    import bass_rust
    from concourse import vector_clock

    f32 = mybir.dt.float32
    fp16 = mybir.dt.float16
    bf16 = mybir.dt.bfloat16
    Exp = mybir.ActivationFunctionType.Exp
    Ln = mybir.ActivationFunctionType.Ln
    mult = mybir.AluOpType.mult
    add = mybir.AluOpType.add

    gps_cast = os.environ.get("KERNEL_NO_GPS_CAST", "") != "1"

    nc = bass.Bass("TRN2", target_bir_lowering=False, debug=False,
                   num_devices=N_CORES)

    xq_d = nc.dram_tensor("xq", [128, KT * S], bf16, kind="ExternalInput").ap()
    xkv_d = nc.dram_tensor("xkv", [128, KT * S], bf16, kind="ExternalInput").ap()
    wq_d = nc.dram_tensor("wq", [128, KT * E], bf16, kind="ExternalInput").ap()
    wk_d = nc.dram_tensor("wk", [128, KT * E], bf16, kind="ExternalInput").ap()
    wv_d = nc.dram_tensor("wv", [128, KT * E], bf16, kind="ExternalInput").ap()
    wo_d = nc.dram_tensor("wo", [128, 2 * D], bf16, kind="ExternalInput").ap()
    cos_d = nc.dram_tensor("cosT", [128, S], bf16, kind="ExternalInput").ap()
    sin_d = nc.dram_tensor("sinT", [128, S], bf16, kind="ExternalInput").ap()
    ones_d = nc.dram_tensor("ones_col", [128, 1], fp16, kind="ExternalInput").ap()
    out = nc.dram_tensor("out", [S, D], fp16, kind="ExternalOutput").ap()

    TC = _fixed_tile_context(tile, bass_rust, vector_clock)

    with TC(nc) as tc:
        with tc.tile_pool(name="persist", bufs=1) as per, \
             tc.tile_pool(name="ps", bufs=1, space="PSUM") as psp, \
             tc.tile_pool(name="edram", bufs=1, space="DRAM") as edr:
            # ---- persistent SBUF ----
            xq_sb = per.tile([128, KT * S], bf16, tag="xq")
            xkv_sb = per.tile([128, KT * S], bf16, tag="xkv")
            wq_sb = per.tile([128, KT * E], bf16, tag="wq")
            wk_sb = per.tile([128, KT * E], bf16, tag="wk")
            wv_sb = per.tile([128, KT * E], bf16, tag="wv")
            wo_sb = per.tile([128, 2 * D], bf16, tag="wo")
            cos_sb = per.tile([128, S], bf16, tag="cos")
            sin_sb = per.tile([128, S], bf16, tag="sin")
            ones_sb = per.tile([128, 1], fp16, tag="ones")
            ones_bf = per.tile([128, 1], bf16, tag="onesb")
            qr_sb = [per.tile([128, S], bf16, tag=f"qr{p}", name=f"qr{p}")
                     for p in range(2)]
            kr_sb = [per.tile([128, S], bf16, tag=f"kr{p}", name=f"kr{p}")
                     for p in range(2)]
            v_sb = per.tile([128, ST * E], bf16, tag="v")
            ctxn_sb = [per.tile([128, S], bf16, tag=f"ctxn{p}", name=f"ctxn{p}")
                       for p in range(2)]
            rb_sb = per.tile([128, S], bf16, tag="rb")
            rsw_sb = per.tile([128, S], bf16, tag="rsw")
            rcos_sb = per.tile([128, S], bf16, tag="rcos")
            e_sb = [per.tile([128, 1024], bf16, tag=f"e{i}", name=f"e{i}")
                    for i in range(8)]
            et_sb = [per.tile([128, 1024], fp16, tag=f"et{p}", name=f"et{p}")
                     for p in range(2)]
            etb_sb = [per.tile([128, 1024], fp16, tag=f"etb{p}", name=f"etb{p}")
                      for p in range(2)]
            etq_sb = [per.tile([128, 1024], fp16, tag=f"etq{p}", name=f"etq{p}")
                      for p in range(2)]
            o_sb = [per.tile([128, D], fp16, tag=f"o{i}", name=f"o{i}")
                    for i in range(2)]
            linv_sb = per.tile([128, 512], f32, tag="linv")
            lbc_sb = [per.tile([128, 512], f32, tag=f"lbc{p}", name=f"lbc{p}")
                      for p in range(2)]
            lscr = edr.tile([4, 512], f32, tag="lscr")

            # ---- PSUM: 8 banks exactly ----
            s_ps = [psp.tile([128, 1024], f32, tag=f"s{i}", name=f"s{i}")
                    for i in range(2)]                       # banks 0-3
            ctx_ps = [psp.tile([128, 512], f32, tag=f"c{p}", name=f"c{p}")
                      for p in range(2)]                     # banks 4-5
            den_ps = psp.tile([128, 512], f32, tag="den")    # bank 6
            rot_ps = psp.tile([128, 512], f32, tag="rot")    # bank 7

            def cast(dst, src):
                nc.vector.tensor_copy(dst, src)

            rope_eng = (nc.gpsimd if os.environ.get("KERNEL_GPS_ROPE", "")
                        == "1" else nc.vector)

            def rope_chunk(dst, c0, ln):
                """dst[:, c0:c0+ln] = rb*cos + swap_halves(rb)*sin over the
                column range [c0, c0+ln) (positions match table columns)."""
                r = slice(c0, c0 + ln)
                for a, bdst in ((0, 32), (32, 0), (64, 96), (96, 64)):
                    nc.sync.dma_start(rsw_sb[bdst:bdst + 32, r],
                                      rb_sb[a:a + 32, r])
                rope_eng.tensor_tensor(rcos_sb[:, r], rb_sb[:, r],
                                       cos_sb[:, r], mult)
                rope_eng.tensor_tensor(rsw_sb[:, r], rsw_sb[:, r],
                                       sin_sb[:, r], mult)
                rope_eng.tensor_tensor(dst[:, r], rcos_sb[:, r],
                                       rsw_sb[:, r], add)

            def proj_mms(ps_tile, pc0, w_sb, x_sb, sc, pcol, n, kts):
                """Accumulate k-tiles kts of W^T[:, pcol:pcol+128] @ x chunk sc
                (n cols) into ps_tile[:, pc0:pc0+n]."""
                for kt in kts:
                    nc.tensor.matmul(
                        ps_tile[:, pc0:pc0 + n],
                        lhsT=w_sb[:, kt * E + pcol: kt * E + pcol + 128],
                        rhs=x_sb[:, sc * 4096 + kt * 512: sc * 4096 + kt * 512 + n],
                        start=(kt == 0), stop=(kt == KT - 1),
                        skip_group_check=True)

            def vproj(st, ps_tile, pc0):
                """v projection of s-tile st into ps_tile[:, pc0:pc0+E]."""
                sc, r = st // 4, (st % 4) * 128
                for kt in range(KT):
                    nc.tensor.matmul(
                        ps_tile[:, pc0:pc0 + E],
                        lhsT=xkv_sb[:, sc * 4096 + kt * 512 + r:
                                    sc * 4096 + kt * 512 + r + 128],
                        rhs=wv_sb[:, kt * E:(kt + 1) * E],
                        start=(kt == 0), stop=(kt == KT - 1),
                        skip_group_check=True)
                cast(v_sb[:, st * E:(st + 1) * E], ps_tile[:, pc0:pc0 + E])

            # ---- startup: staged loads interleaved with all projections ----
            # k chunks rotate through s_ps[0] halves, v tiles through s_ps[1]
            # halves, q chunk 0 uses the ctx banks. Emission order = PE order;
            # each stage's DMAs are emitted just before their consumers.
            nc.vector.memset(ones_bf[:, :], 1.0)

            def kp_chunk(sc, p):
                pc0 = (p % 2) * 512
                proj_mms(s_ps[0], pc0, wk_sb, xkv_sb, sc, p * 128, 512,
                         range(KT))
                c0 = sc * 512
                cast(rb_sb[:, c0:c0 + 512], s_ps[0][:, pc0:pc0 + 512])
                rope_chunk(kr_sb[p], c0, 512)

            nc.sync.dma_start(wk_sb[:, :], wk_d[:, :])
            nc.sync.dma_start(xkv_sb[:, 0:4096], xkv_d[:, 0:4096])
            nc.gpsimd.dma_start(cos_sb[:, :], cos_d[:, :])
            nc.gpsimd.dma_start(sin_sb[:, :], sin_d[:, :])
            nc.sync.dma_start(wv_sb[:, :], wv_d[:, :])
            nc.sync.dma_start(xkv_sb[:, 4096:8192], xkv_d[:, 4096:8192])
            kp_chunk(0, 0)
            kp_chunk(0, 1)
            nc.sync.dma_start(wq_sb[:, :], wq_d[:, :])
            nc.sync.dma_start(xq_sb[:, 0:4096], xq_d[:, 0:4096])
            nc.gpsimd.dma_start(wo_sb[:, :], wo_d[:, :])
            nc.gpsimd.dma_start(ones_sb[:, :], ones_d[:, :])
            for sc in range(1, QC):
                nc.gpsimd.dma_start(xq_sb[:, sc * 4096:(sc + 1) * 4096],
                                    xq_d[:, sc * 4096:(sc + 1) * 4096])
            for st in range(4):
                vproj(st, s_ps[1], (st % 2) * 512)
            kp_chunk(1, 0)
            kp_chunk(1, 1)
            for p in range(2):
                proj_mms(ctx_ps[p], 0, wq_sb, xq_sb, 0, p * 128, 512,
                         range(KT))
                cast(rb_sb[:, 0:512], ctx_ps[p][:, 0:512])
                rope_chunk(qr_sb[p], 0, 512)
            for st in range(4, 8):
                vproj(st, s_ps[1], (st % 2) * 512)
            for sc in range(2, QC):
                nc.sync.dma_start(xkv_sb[:, sc * 4096:(sc + 1) * 4096],
                                  xkv_d[:, sc * 4096:(sc + 1) * 4096])
                kp_chunk(sc, 0)
                kp_chunk(sc, 1)
                for st in range(4 * sc, 4 * sc + 4):
                    vproj(st, s_ps[1], (st % 2) * 512)

            # ---- window filler items -------------------------------------
            def qproj_items(qh):
                def it(p, qh=qh):
                    proj_mms(rot_ps, 0, wq_sb, xq_sb, qh, p * 128, 512,
                             range(KT))
                    c0 = qh * 512
                    cast(rb_sb[:, c0:c0 + 512], rot_ps[:, 0:512])
                    rope_chunk(qr_sb[p], c0, 512)
                return [lambda p=p: it(p) for p in range(2)]

            def oproj_items(qh, banks=None, act_cast=False):
                """Output projection of window qh's s-tiles. banks: list of
                (tile, col0) PSUM regions to rotate through (default rot).
                act_cast: alternate casts DVE/ACT (tail only, ACT idle)."""
                if banks is None:
                    banks = [(rot_ps, 0)]
                items = []
                for j, st in enumerate(range(qh * 4, qh * 4 + 4)):
                    for ch in range(2):
                        bt, bc = banks[(2 * j + ch) % len(banks)]
                        def it(st=st, ch=ch, bt=bt, bc=bc):
                            for p in range(2):
                                nc.tensor.matmul(
                                    bt[:, bc:bc + 512],
                                    lhsT=ctxn_sb[p][:, st * 128:(st + 1) * 128],
                                    rhs=wo_sb[:, p * D + ch * 512:
                                              p * D + ch * 512 + 512],
                                    start=(p == 0), stop=(p == 1),
                                    skip_group_check=True)
                            ob = o_sb[st % 2]
                            if act_cast and ch == 1:
                                nc.scalar.copy(ob[:, ch * 512:(ch + 1) * 512],
                                               bt[:, bc:bc + 512])
                            else:
                                cast(ob[:, ch * 512:(ch + 1) * 512],
                                     bt[:, bc:bc + 512])
                            if ch == 1:
                                nc.sync.dma_start(
                                    out[st * 128:(st + 1) * 128, :], ob[:, :])
                        items.append(it)
                return items

            # ---- SDPA windows ----
            pair_eng = (nc.gpsimd if os.environ.get("KERNEL_GPS_PAIR", "")
                        == "1" else nc.vector)
            uglob = 0
            carry = {}   # units of the next window emitted early (prologue)

            def emit_unit(qh, u):
                ki, p = u // 2, u % 2
                qs_ = slice(qh * 512, (qh + 1) * 512)
                sb = s_ps[u % 2]
                nc.tensor.matmul(
                    sb[:, 0:512],
                    lhsT=kr_sb[p][0:64, ki * 128:(ki + 1) * 128],
                    rhs=qr_sb[p][0:64, qs_],
                    tile_position=(0, 0), start=True, stop=True)
                nc.tensor.matmul(
                    sb[:, 512:1024],
                    lhsT=kr_sb[p][64:128, ki * 128:(ki + 1) * 128],
                    rhs=qr_sb[p][64:128, qs_],
                    tile_position=(64, 0), start=True, stop=True)
                e = e_sb[emit_unit.uglob % 8]
                emit_unit.uglob += 1
                nc.scalar.activation(e[:, :], sb[:, :], Exp)
                return e
            emit_unit.uglob = 0

            for qh in range(QC):
                qs = slice(qh * 512, (qh + 1) * 512)
                # (due_u, fn): emitted when u >= due_u, strict FIFO
                sched = []
                if qh > 0:
                    its = oproj_items(qh - 1)
                    for j, it in enumerate(its[:4]):
                        sched.append((2 + 3 * j, it))
                    for j, it in enumerate(its[4:]):
                        sched.append((18 + 3 * j, it))
                if qh < QC - 1:
                    for j, it in enumerate(qproj_items(qh + 1)):
                        sched.append((12 + 3 * j, it))
                sched.sort(key=lambda t: t[0])

                nc.vector.memset(den_ps[:, :], 1.0)

                def emit_ctx(u, e):
                    ki, p = u // 2, u % 2
                    for half in range(2):
                        nc.tensor.matmul(
                            ctx_ps[p][half * 64:(half + 1) * 64, :],
                            lhsT=v_sb[:, ki * E + (2 * p + half) * 64:
                                      ki * E + (2 * p + half) * 64 + 64],
                            rhs=e[:, half * 512:(half + 1) * 512],
                            tile_position=(0, half * 64),
                            start=(ki == 0), stop=(ki == ST - 1),
                            skip_group_check=True)

                def emit_den(u, src, start, stop):
                    # 2 ones-matmuls accumulating src's halves into the den
                    # rows of unit u's head pair
                    p = u % 2
                    for half in range(2):
                        g = 2 * p + half
                        nc.tensor.matmul(
                            den_ps[g * 32: g * 32 + 1, :],
                            lhsT=ones_sb[:, :],
                            rhs=src[:, half * 512:(half + 1) * 512],
                            tile_position=(0, g * 32),
                            start=start, stop=stop,
                            skip_group_check=True)

                e_of_u = carry
                carry = {}
                si = 0
                for u in range(32):
                    ki, p = u // 2, u % 2
                    if u in e_of_u:
                        e = e_of_u[u]       # emitted in previous window's tail
                    else:
                        e = emit_unit(qh, u)
                        e_of_u[u] = e
                    # deferred den matmuls (emitted BEFORE the pair-sum that
                    # would overwrite their source): quads for ki 3/7/11 on
                    # the gpsimd-combined tile, pair path for ki 13
                    if u >= 12 and (u - 6) // 2 in (3, 7, 11):
                        emit_den(u - 6, etq_sb[(u - 6) % 2],
                                 start=((u - 6) // 2 == 3), stop=False)
                    if u >= 4 and (u - 4) // 2 == 13:
                        emit_den(u - 4, et_sb[(u - 4) % 2],
                                 start=False, stop=False)
                    if ki % 2 == 1:
                        # pair-sum exp tiles into fp16 (exact: values < 600);
                        # pairs ki%4==1 and ki>=13 land in et, ki 3/7/11 in
                        # etb, then gpsimd combines et+etb into the quad tile
                        dst = (et_sb[p] if (ki % 4 == 1 or ki >= 13)
                               else etb_sb[p])
                        pair_eng.tensor_tensor(dst[:, :],
                                               e_of_u[u - 2][:, :],
                                               e_of_u[u][:, :], add)
                        if ki in (3, 7, 11):
                            nc.gpsimd.tensor_tensor(etq_sb[p][:, :],
                                                    et_sb[p][:, :],
                                                    etb_sb[p][:, :], add)
                    while si < len(sched) and sched[si][0] <= u:
                        sched[si][1]()
                        si += 1
                    # ctx lags 3 units so a window's first ctx (which waits on
                    # the previous normalize chain) doesn't stall the PE queue
                    if u >= 3:
                        emit_ctx(u - 3, e_of_u[u - 3])
                emit_ctx(29, e_of_u[29])
                emit_ctx(30, e_of_u[30])
                emit_ctx(31, e_of_u[31])
                emit_den(30, et_sb[0], start=False, stop=True)
                emit_den(31, et_sb[1], start=False, stop=True)
                while si < len(sched):
                    sched[si][1]()
                    si += 1

                # prologue: next window's first units keep the exp stream hot
                # while this window's normalize chain runs
                if qh < QC - 1:
                    for un in range(2):
                        carry[un] = emit_unit(qh + 1, un)

                # normalize: linv rows -> DRAM roundtrip broadcast -> ctx*linv
                nc.vector.reciprocal(linv_sb[:, :], den_ps[:, :])
                nc.sync.dma_start(lscr[:, :], linv_sb[0:128:32, :])
                for g, (p, half) in enumerate(((0, 0), (0, 1), (1, 0), (1, 1))):
                    nc.sync.dma_start(
                        lbc_sb[p][half * 64:(half + 1) * 64, :],
                        lscr[g:g + 1, :].partition_broadcast(64))
                for p in range(2):
                    nc.vector.tensor_tensor(
                        ctxn_sb[p][:, qs], ctx_ps[p][:, :], lbc_sb[p][:, :], mult)

            # tail: output projection of the last window on freed score banks
            for it in oproj_items(QC - 1, banks=[(s_ps[0], 0), (s_ps[0], 512),
                                                 (s_ps[1], 0), (s_ps[1], 512)],
                                  act_cast=True):
                it()

    if split_waits:
        _split_multiwait_instructions(nc, mybir, bass_rust.SyncInfo)
    return nc


def kernel(x_q, x_kv, wq, bq, wk, bk, wv, bv, wo, bo):
    from concourse import bass_utils

    x_q = np.asarray(x_q, dtype=np.float32)
    x_kv = np.asarray(x_kv, dtype=np.float32)
    wq = np.asarray(wq, dtype=np.float32); bq = np.asarray(bq, dtype=np.float32)
    wk = np.asarray(wk, dtype=np.float32); bk = np.asarray(bk, dtype=np.float32)
    wv = np.asarray(wv, dtype=np.float32); bv = np.asarray(bv, dtype=np.float32)
    wo = np.asarray(wo, dtype=np.float32); bo = np.asarray(bo, dtype=np.float32)

    assert not (np.any(bq) or np.any(bk) or np.any(bv)), \
        "nonzero qkv biases not supported by this build"

    in_maps = _host_prep(x_q, x_kv, wq, bq, wk, bk, wv, bv, wo)

    if "prog" not in _PROGRAM_CACHE:
        _PROGRAM_CACHE["prog"] = build_program()
    nc = _PROGRAM_CACHE["prog"]

    res = bass_utils.run_bass_kernel_spmd(
        nc, in_maps, core_ids=list(range(N_CORES)),
        trace=os.environ.get("KERNEL_TRACE", "") == "1",
        tmpdir=os.environ.get("KERNEL_TRACE_DIR") or None)
    _PROGRAM_CACHE["last_result"] = res

    out = np.zeros((B, S, D), dtype=np.float32)
    for c in range(N_CORES):
        out[c // QUADS] += np.asarray(res.results[c]["out"], dtype=np.float32)
    out += bo[None, None, :]
    return out


# revision 53
# speedup vs baseline: 1.0578x; 1.0578x over previous
"""Trainium2 Bass kernel for nn_CrossAttentionBlock (B=2, S=2048, D=1024, H=16, HD=64).

Sharding: 8 cores = 2 batches x 4 head-quads (4 heads each, E=256 channels).
Each core computes q/k/v projections for its quad, RoPE, SDPA, and a partial
output projection [S, D] (fp16); host sums the 4 partials per batch + bo.

Software-pipelined single pass per core. The scalar engine (exp, 128
[128,1024] tiles, ~1.1us each) and the tensor engine (~960 matmuls at
~0.4us issue cost each) are both near-saturated; everything else hides
under them:
  - startup is minimal: k-projection chunk 0, v s-tiles 0..3, q-projection
    of window 0. The rest of k/v projection enters window 0 as
    deadline-scheduled PE filler items, so the exp stream starts ~20us in.
  - 4 SDPA windows (512 q each), 32 units each (16 k-tiles x 2 head-pairs):
    scores pair (row-tiled) -> exp (double-buffered scores PSUM) -> ctx pair
    (col-tiled, PSUM accum). Denominators: exp tiles are pair-summed on DVE
    into fp16 (exact at these magnitudes), halving the ones-matmul count.
  - PSUM->SBUF casts run on GpSimd, rope swaps on the sync queue, output
    stores + normalize broadcast DMAs on GpSimd: DVE keeps only rope
    multiplies, pair-sums, normalize, reciprocal.
  - RoPE: even/odd permutation folded into w_q/w_k rows host-side ->
    rot-half rope out = q*cos + swap_halves(q)*sin_signed.
"""
import os
import sys

sys.path.insert(0, "/opt/trn_rl_repo")

import numpy as np
import ml_dtypes

BF16 = ml_dtypes.bfloat16

B, S, D, H = 2, 2048, 1024, 16
HD = D // H          # 64
DIM = HD // 2        # 32
QUADS = 4            # head groups of 4
E = D // QUADS       # 256 channels per core
ROPE_BASE = 10000.0
N_CORES = 8

KT = D // 128        # 8 k-tiles of the contraction dim
ST = S // 128        # 16 s-tiles
QC = S // 512        # 4 q-chunks (SDPA windows)


def _host_prep(x_q, x_kv, wq, bq, wk, bk, wv, bv, wo):
    """Per-core input maps, every tensor already in its SBUF layout."""
    perm = np.concatenate([np.arange(0, HD, 2), np.arange(1, HD, 2)])  # even|odd
    scale = 1.0 / np.sqrt(HD)

    freqs = np.exp(-np.arange(DIM, dtype=np.float64) * np.log(ROPE_BASE) / DIM)
    ang = np.arange(S, dtype=np.float64)[:, None] * freqs[None, :]     # [S, 32]
    cos = np.cos(ang).T                                                # [32, S]
    sin = np.sin(ang).T
    cos64 = np.concatenate([cos, cos], axis=0)                         # [64, S]
    sin64 = np.concatenate([-sin, sin], axis=0)
    cosT = np.concatenate([cos64, cos64], axis=0).astype(BF16)         # [128, S]
    sinT = np.concatenate([sin64, sin64], axis=0).astype(BF16)

    def x_layout(x):
        # x [S, D] -> xT [D, S] -> sc-major SBUF layout [128, 4*4096]:
        # chunk sc at cols sc*4096, inside: k-tile kt at +kt*512
        xT = x.T.reshape(KT, 128, QC, 512)
        return np.ascontiguousarray(
            xT.transpose(1, 2, 0, 3).reshape(128, KT * S)).astype(BF16)

    def w_layout(w, permute, s):
        # quad rows [256, 1024] (maybe permuted per head, scaled) -> d-major
        # [1024, 256] -> SBUF [128, 8*256] (k-tile kt at cols kt*256)
        blocks = []
        for h in range(4):
            wb = w[h * HD:(h + 1) * HD, :]
            if permute:
                wb = wb[perm, :]
            blocks.append(wb * s)
        wT = np.concatenate(blocks, axis=0).T                          # [1024, 256]
        return np.ascontiguousarray(
            wT.reshape(KT, 128, E).transpose(1, 0, 2).reshape(128, KT * E)
        ).astype(BF16)

    in_maps = []
    for c in range(N_CORES):
        b_ = c // QUADS
        g = c % QUADS
        hs = slice(g * E, (g + 1) * E)
        woT = wo[:, hs].T                                              # [256, 1024]
        wo_dev = np.ascontiguousarray(
            woT.reshape(2, 128, D).transpose(1, 0, 2).reshape(128, 2 * D)
        ).astype(BF16)
        in_maps.append({
            "xq": x_layout(x_q[b_]), "xkv": x_layout(x_kv[b_]),
            "wq": w_layout(wq[hs, :], True, scale),
            "wk": w_layout(wk[hs, :], True, 1.0),
            "wv": w_layout(wv[hs, :], False, 1.0),
            "wo": wo_dev,
            "cosT": np.ascontiguousarray(cosT),
            "sinT": np.ascontiguousarray(sinT),
            "ones_col": np.ones((128, 1), dtype=np.float16),
        })
    return in_maps


# ---------------------------------------------------------------------------
_PROGRAM_CACHE = {}


def _fixed_tile_context(tile_mod, bass_rust_mod, vector_clock_mod):
    """TileContext whose tail drain splits multi-sem waits into single-wait
    NOPs (this walrus rejects >1 sync-wait on one instruction)."""
    SyncInfo = bass_rust_mod.SyncInfo
    ScopedClock = vector_clock_mod.ScopedClock

    class TC(tile_mod.TileContext):
        def _drain_and_barrier(self, tick_clock, wait_clock):
            harvest = self.nc.sync.nop(nofuse=True)
            wait_clock.add_sem_waits(
                harvest.ins, ScopedClock({None: tick_clock.global_clock}))
            si = harvest.ins.sync_info
            waits = list(si.on_wait) if si is not None else []
            if len(waits) > 1:
                harvest.ins.sync_info = SyncInfo(
                    on_wait=[waits[0]], on_update=list(si.on_update))
                for w in waits[1:]:
                    nop = self.nc.sync.nop(nofuse=True)
                    nop.ins.sync_info = SyncInfo(on_wait=[w], on_update=[])
            self.nc.sync.drain()
            self.nc.all_engine_barrier()
            assert self.sems is not None
            popped = self.nc._tile_sem_poison_stack.pop()
            assert popped is self._sem_poison
            self.nc.clear_and_free_semaphores(list(self.sems.allocated().values()))
            self.nc.all_engine_barrier()

    return TC


def _split_multiwait_instructions(nc, mybir, SyncInfo):
    """This walrus build rejects >1 sync-wait per instruction; hoist extra
    waits onto single-wait NOPs inserted just before, on the same engine."""
    ctr = 0
    for blk in nc.m.functions[0].blocks:
        insts = blk.instructions
        i = 0
        while i < len(insts):
            inst = insts[i]
            si = inst.sync_info
            if si is not None and len(si.on_wait) > 1:
                waits = list(si.on_wait)
                inst.sync_info = SyncInfo(on_wait=[waits[-1]],
                                          on_update=list(si.on_update))
                nops = []
                for w in waits[:-1]:
                    nop = mybir.InstNoOp(name=f"waitsplit_{ctr}", ins=[], outs=[])
                    ctr += 1
                    nop.engine = inst.engine
                    nop.sync_info = SyncInfo(on_wait=[w], on_update=[])
                    nops.append(nop)
                insts[i:i] = nops
                i += len(nops)
            i += 1
    return ctr


def build_program(split_waits=True):
    import concourse.bass as bass
    import concourse.mybir as mybir
    import concourse.tile as tile
    import bass_rust
    from concourse import vector_clock

    f32 = mybir.dt.float32
    fp16 = mybir.dt.float16
    bf16 = mybir.dt.bfloat16
    Exp = mybir.ActivationFunctionType.Exp
    Ln = mybir.ActivationFunctionType.Ln
    mult = mybir.AluOpType.mult
    add = mybir.AluOpType.add

    gps_cast = os.environ.get("KERNEL_NO_GPS_CAST", "") != "1"

    nc = bass.Bass("TRN2", target_bir_lowering=False, debug=False,
                   num_devices=N_CORES)

    xq_d = nc.dram_tensor("xq", [128, KT * S], bf16, kind="ExternalInput").ap()
    xkv_d = nc.dram_tensor("xkv", [128, KT * S], bf16, kind="ExternalInput").ap()
    wq_d = nc.dram_tensor("wq", [128, KT * E], bf16, kind="ExternalInput").ap()
    wk_d = nc.dram_tensor("wk", [128, KT * E], bf16, kind="ExternalInput").ap()
    wv_d = nc.dram_tensor("wv", [128, KT * E], bf16, kind="ExternalInput").ap()
    wo_d = nc.dram_tensor("wo", [128, 2 * D], bf16, kind="ExternalInput").ap()
    cos_d = nc.dram_tensor("cosT", [128, S], bf16, kind="ExternalInput").ap()
    sin_d = nc.dram_tensor("sinT", [128, S], bf16, kind="ExternalInput").ap()
    ones_d = nc.dram_tensor("ones_col", [128, 1], fp16, kind="ExternalInput").ap()
    out = nc.dram_tensor("out", [S, D], fp16, kind="ExternalOutput").ap()

    TC = _fixed_tile_context(tile, bass_rust, vector_clock)

    with TC(nc) as tc:
        with tc.tile_pool(name="persist", bufs=1) as per, \
             tc.tile_pool(name="ps", bufs=1, space="PSUM") as psp, \
             tc.tile_pool(name="edram", bufs=1, space="DRAM") as edr:
            # ---- persistent SBUF ----
            xq_sb = per.tile([128, KT * S], bf16, tag="xq")
            xkv_sb = per.tile([128, KT * S], bf16, tag="xkv")
            wq_sb = per.tile([128, KT * E], bf16, tag="wq")
            wk_sb = per.tile([128, KT * E], bf16, tag="wk")
            wv_sb = per.tile([128, KT * E], bf16, tag="wv")
            wo_sb = per.tile([128, 2 * D], bf16, tag="wo")
            cos_sb = per.tile([128, S], bf16, tag="cos")
            sin_sb = per.tile([128, S], bf16, tag="sin")
            ones_sb = per.tile([128, 1], fp16, tag="ones")
            ones_bf = per.tile([128, 1], bf16, tag="onesb")
            qr_sb = [per.tile([128, S], bf16, tag=f"qr{p}", name=f"qr{p}")
                     for p in range(2)]
            kr_sb = [per.tile([128, S], bf16, tag=f"kr{p}", name=f"kr{p}")
                     for p in range(2)]
            v_sb = per.tile([128, ST * E], bf16, tag="v")
            ctxn_sb = [per.tile([128, S], bf16, tag=f"ctxn{p}", name=f"ctxn{p}")
                       for p in range(2)]
            rb_sb = per.tile([128, S], bf16, tag="rb")
            rsw_sb = per.tile([128, S], bf16, tag="rsw")
            rcos_sb = per.tile([128, S], bf16, tag="rcos")
            e_sb = [per.tile([128, 1024], bf16, tag=f"e{i}", name=f"e{i}")
                    for i in range(8)]
            et_sb = [per.tile([128, 1024], fp16, tag=f"et{p}", name=f"et{p}")
                     for p in range(2)]
            o_sb = [per.tile([128, D], fp16, tag=f"o{i}", name=f"o{i}")
                    for i in range(2)]
            linv_sb = per.tile([128, 512], f32, tag="linv")
            lbc_sb = [per.tile([128, 512], f32, tag=f"lbc{p}", name=f"lbc{p}")
                      for p in range(2)]
            lscr = edr.tile([4, 512], f32, tag="lscr")

            # ---- PSUM: 8 banks exactly ----
            s_ps = [psp.tile([128, 1024], f32, tag=f"s{i}", name=f"s{i}")
                    for i in range(2)]                       # banks 0-3
            ctx_ps = [psp.tile([128, 512], f32, tag=f"c{p}", name=f"c{p}")
                      for p in range(2)]                     # banks 4-5
            den_ps = psp.tile([128, 512], f32, tag="den")    # bank 6
            rot_ps = psp.tile([128, 512], f32, tag="rot")    # bank 7

            def cast(dst, src):
                nc.vector.tensor_copy(dst, src)

            rope_eng = (nc.gpsimd if os.environ.get("KERNEL_GPS_ROPE", "")
                        == "1" else nc.vector)

            def rope_chunk(dst, c0, ln):
                """dst[:, c0:c0+ln] = rb*cos + swap_halves(rb)*sin over the
                column range [c0, c0+ln) (positions match table columns)."""
                r = slice(c0, c0 + ln)
                for a, bdst in ((0, 32), (32, 0), (64, 96), (96, 64)):
                    nc.sync.dma_start(rsw_sb[bdst:bdst + 32, r],
                                      rb_sb[a:a + 32, r])
                rope_eng.tensor_tensor(rcos_sb[:, r], rb_sb[:, r],
                                       cos_sb[:, r], mult)
                rope_eng.tensor_tensor(rsw_sb[:, r], rsw_sb[:, r],
                                       sin_sb[:, r], mult)
                rope_eng.tensor_tensor(dst[:, r], rcos_sb[:, r],
                                       rsw_sb[:, r], add)

            def proj_mms(ps_tile, pc0, w_sb, x_sb, sc, pcol, n, kts):
                """Accumulate k-tiles kts of W^T[:, pcol:pcol+128] @ x chunk sc
                (n cols) into ps_tile[:, pc0:pc0+n]."""
                for kt in kts:
                    nc.tensor.matmul(
                        ps_tile[:, pc0:pc0 + n],
                        lhsT=w_sb[:, kt * E + pcol: kt * E + pcol + 128],
                        rhs=x_sb[:, sc * 4096 + kt * 512: sc * 4096 + kt * 512 + n],
                        start=(kt == 0), stop=(kt == KT - 1),
                        skip_group_check=True)

            def vproj(st, ps_tile, pc0):
                """v projection of s-tile st into ps_tile[:, pc0:pc0+E]."""
                sc, r = st // 4, (st % 4) * 128
                for kt in range(KT):
                    nc.tensor.matmul(
                        ps_tile[:, pc0:pc0 + E],
                        lhsT=xkv_sb[:, sc * 4096 + kt * 512 + r:
                                    sc * 4096 + kt * 512 + r + 128],
                        rhs=wv_sb[:, kt * E:(kt + 1) * E],
                        start=(kt == 0), stop=(kt == KT - 1),
                        skip_group_check=True)
                cast(v_sb[:, st * E:(st + 1) * E], ps_tile[:, pc0:pc0 + E])

            # ---- startup: staged loads interleaved with all projections ----
            # k chunks rotate through s_ps[0] halves, v tiles through s_ps[1]
            # halves, q chunk 0 uses the ctx banks. Emission order = PE order;
            # each stage's DMAs are emitted just before their consumers.
            nc.vector.memset(ones_bf[:, :], 1.0)

            def kp_chunk(sc, p):
                pc0 = (p % 2) * 512
                proj_mms(s_ps[0], pc0, wk_sb, xkv_sb, sc, p * 128, 512,
                         range(KT))
                c0 = sc * 512
                cast(rb_sb[:, c0:c0 + 512], s_ps[0][:, pc0:pc0 + 512])
                rope_chunk(kr_sb[p], c0, 512)

            nc.sync.dma_start(wk_sb[:, :], wk_d[:, :])
            nc.sync.dma_start(xkv_sb[:, 0:4096], xkv_d[:, 0:4096])
            nc.gpsimd.dma_start(cos_sb[:, :], cos_d[:, :])
            nc.gpsimd.dma_start(sin_sb[:, :], sin_d[:, :])
            nc.sync.dma_start(wv_sb[:, :], wv_d[:, :])
            nc.sync.dma_start(xkv_sb[:, 4096:8192], xkv_d[:, 4096:8192])
            kp_chunk(0, 0)
            kp_chunk(0, 1)
            nc.sync.dma_start(wq_sb[:, :], wq_d[:, :])
            nc.sync.dma_start(xq_sb[:, 0:4096], xq_d[:, 0:4096])
            nc.gpsimd.dma_start(wo_sb[:, :], wo_d[:, :])
            nc.gpsimd.dma_start(ones_sb[:, :], ones_d[:, :])
            for sc in range(1, QC):
                nc.gpsimd.dma_start(xq_sb[:, sc * 4096:(sc + 1) * 4096],
                                    xq_d[:, sc * 4096:(sc + 1) * 4096])
            for st in range(4):
                vproj(st, s_ps[1], (st % 2) * 512)
            kp_chunk(1, 0)
            kp_chunk(1, 1)
            for p in range(2):
                proj_mms(ctx_ps[p], 0, wq_sb, xq_sb, 0, p * 128, 512,
                         range(KT))
                cast(rb_sb[:, 0:512], ctx_ps[p][:, 0:512])
                rope_chunk(qr_sb[p], 0, 512)
            for st in range(4, 8):
                vproj(st, s_ps[1], (st % 2) * 512)
            for sc in range(2, QC):
                nc.sync.dma_start(xkv_sb[:, sc * 4096:(sc + 1) * 4096],
                                  xkv_d[:, sc * 4096:(sc + 1) * 4096])
                kp_chunk(sc, 0)
                kp_chunk(sc, 1)
                for st in range(4 * sc, 4 * sc + 4):
                    vproj(st, s_ps[1], (st % 2) * 512)

            # ---- window filler items -------------------------------------
            def qproj_items(qh):
                def it(p, qh=qh):
                    proj_mms(rot_ps, 0, wq_sb, xq_sb, qh, p * 128, 512,
                             range(KT))
                    c0 = qh * 512
                    cast(rb_sb[:, c0:c0 + 512], rot_ps[:, 0:512])
                    rope_chunk(qr_sb[p], c0, 512)
                return [lambda p=p: it(p) for p in range(2)]

            def oproj_items(qh, banks=None, act_cast=False):
                """Output projection of window qh's s-tiles. banks: list of
                (tile, col0) PSUM regions to rotate through (default rot).
                act_cast: alternate casts DVE/ACT (tail only, ACT idle)."""
                if banks is None:
                    banks = [(rot_ps, 0)]
                items = []
                for j, st in enumerate(range(qh * 4, qh * 4 + 4)):
                    for ch in range(2):
                        bt, bc = banks[(2 * j + ch) % len(banks)]
                        def it(st=st, ch=ch, bt=bt, bc=bc):
                            for p in range(2):
                                nc.tensor.matmul(
                                    bt[:, bc:bc + 512],
                                    lhsT=ctxn_sb[p][:, st * 128:(st + 1) * 128],
                                    rhs=wo_sb[:, p * D + ch * 512:
                                              p * D + ch * 512 + 512],
                                    start=(p == 0), stop=(p == 1),
                                    skip_group_check=True)
                            ob = o_sb[st % 2]
                            if act_cast and ch == 1:
                                nc.scalar.copy(ob[:, ch * 512:(ch + 1) * 512],
                                               bt[:, bc:bc + 512])
                            else:
                                cast(ob[:, ch * 512:(ch + 1) * 512],
                                     bt[:, bc:bc + 512])
                            if ch == 1:
                                nc.sync.dma_start(
                                    out[st * 128:(st + 1) * 128, :], ob[:, :])
                        items.append(it)
                return items

            # ---- SDPA windows ----
            pair_eng = (nc.gpsimd if os.environ.get("KERNEL_GPS_PAIR", "")
                        == "1" else nc.vector)
            uglob = 0
            carry = {}   # units of the next window emitted early (prologue)

            def emit_unit(qh, u):
                ki, p = u // 2, u % 2
                qs_ = slice(qh * 512, (qh + 1) * 512)
                sb = s_ps[u % 2]
                nc.tensor.matmul(
                    sb[:, 0:512],
                    lhsT=kr_sb[p][0:64, ki * 128:(ki + 1) * 128],
                    rhs=qr_sb[p][0:64, qs_],
                    tile_position=(0, 0), start=True, stop=True)
                nc.tensor.matmul(
                    sb[:, 512:1024],
                    lhsT=kr_sb[p][64:128, ki * 128:(ki + 1) * 128],
                    rhs=qr_sb[p][64:128, qs_],
                    tile_position=(64, 0), start=True, stop=True)
                e = e_sb[emit_unit.uglob % 8]
                emit_unit.uglob += 1
                nc.scalar.activation(e[:, :], sb[:, :], Exp)
                return e
            emit_unit.uglob = 0

            for qh in range(QC):
                qs = slice(qh * 512, (qh + 1) * 512)
                # (due_u, fn): emitted when u >= due_u, strict FIFO
                sched = []
                if qh > 0:
                    its = oproj_items(qh - 1)
                    for j, it in enumerate(its[:4]):
                        sched.append((2 + 3 * j, it))
                    for j, it in enumerate(its[4:]):
                        sched.append((18 + 3 * j, it))
                if qh < QC - 1:
                    for j, it in enumerate(qproj_items(qh + 1)):
                        sched.append((12 + 3 * j, it))
                sched.sort(key=lambda t: t[0])

                nc.vector.memset(den_ps[:, :], 1.0)

                def emit_ctx(u, e):
                    ki, p = u // 2, u % 2
                    for half in range(2):
                        nc.tensor.matmul(
                            ctx_ps[p][half * 64:(half + 1) * 64, :],
                            lhsT=v_sb[:, ki * E + (2 * p + half) * 64:
                                      ki * E + (2 * p + half) * 64 + 64],
                            rhs=e[:, half * 512:(half + 1) * 512],
                            tile_position=(0, half * 64),
                            start=(ki == 0), stop=(ki == ST - 1),
                            skip_group_check=True)

                def emit_den(u, direct=None):
                    # den matmuls for the pair completed at unit u (ki odd),
                    # deferred 2 slots so the pair-sum engine has slack.
                    # direct: list of e tiles to sum individually instead of
                    # via the pair tile (used for the final pair: shortens the
                    # window-end exp->TT->den->reciprocal critical chain)
                    ki, p = u // 2, u % 2
                    srcs = [et_sb[p]] if direct is None else direct
                    ones = ones_sb if direct is None else ones_bf
                    for j, src in enumerate(srcs):
                        for half in range(2):
                            g = 2 * p + half
                            nc.tensor.matmul(
                                den_ps[g * 32: g * 32 + 1, :],
                                lhsT=ones[:, :],
                                rhs=src[:, half * 512:(half + 1) * 512],
                                tile_position=(0, g * 32),
                                start=(ki == 1 and j == 0),
                                stop=(ki == ST - 1 and j == len(srcs) - 1),
                                skip_group_check=True)

                e_of_u = carry
                carry = {}
                si = 0
                for u in range(32):
                    ki, p = u // 2, u % 2
                    if u in e_of_u:
                        e = e_of_u[u]       # emitted in previous window's tail
                    else:
                        e = emit_unit(qh, u)
                        e_of_u[u] = e
                    if u >= 4 and ((u - 4) // 2) % 2 == 1:
                        # deferred den matmuls read et BEFORE this unit's
                        # pair-sum overwrites it
                        emit_den(u - 4)
                    if ki % 2 == 1:
                        # pair-sum exp tiles into fp16 (exact: values < 600)
                        pair_eng.tensor_tensor(et_sb[p][:, :],
                                               e_of_u[u - 2][:, :],
                                               e_of_u[u][:, :], add)
                    while si < len(sched) and sched[si][0] <= u:
                        sched[si][1]()
                        si += 1
                    # ctx lags 3 units so a window's first ctx (which waits on
                    # the previous normalize chain) doesn't stall the PE queue
                    if u >= 3:
                        emit_ctx(u - 3, e_of_u[u - 3])
                emit_ctx(29, e_of_u[29])
                emit_ctx(30, e_of_u[30])
                emit_ctx(31, e_of_u[31])
                emit_den(30)
                emit_den(31)
                while si < len(sched):
                    sched[si][1]()
                    si += 1

                # prologue: next window's first units keep the exp stream hot
                # while this window's normalize chain runs
                if qh < QC - 1:
                    for un in range(2):
                        carry[un] = emit_unit(qh + 1, un)

                # normalize: linv rows -> DRAM roundtrip broadcast -> ctx*linv
                nc.vector.reciprocal(linv_sb[:, :], den_ps[:, :])
                nc.sync.dma_start(lscr[:, :], linv_sb[0:128:32, :])
                for g, (p, half) in enumerate(((0, 0), (0, 1), (1, 0), (1, 1))):
                    nc.sync.dma_start(
                        lbc_sb[p][half * 64:(half + 1) * 64, :],
                        lscr[g:g + 1, :].partition_broadcast(64))
                for p in range(2):
                    nc.vector.tensor_tensor(
                        ctxn_sb[p][:, qs], ctx_ps[p][:, :], lbc_sb[p][:, :], mult)

            # tail: output projection of the last window on freed score banks
            for it in oproj_items(QC - 1, banks=[(s_ps[0], 0), (s_ps[0], 512),
                                                 (s_ps[1], 0), (s_ps[1], 512)],
                                  act_cast=True):
                it()

    if split_waits:
        _split_multiwait_instructions(nc, mybir, bass_rust.SyncInfo)
    return nc


def kernel(x_q, x_kv, wq, bq, wk, bk, wv, bv, wo, bo):
    from concourse import bass_utils

    x_q = np.asarray(x_q, dtype=np.float32)
    x_kv = np.asarray(x_kv, dtype=np.float32)
    wq = np.asarray(wq, dtype=np.float32); bq = np.asarray(bq, dtype=np.float32)
    wk = np.asarray(wk, dtype=np.float32); bk = np.asarray(bk, dtype=np.float32)
    wv = np.asarray(wv, dtype=np.float32); bv = np.asarray(bv, dtype=np.float32)
    wo = np.asarray(wo, dtype=np.float32); bo = np.asarray(bo, dtype=np.float32)

    assert not (np.any(bq) or np.any(bk) or np.any(bv)), \
        "nonzero qkv biases not supported by this build"

    in_maps = _host_prep(x_q, x_kv, wq, bq, wk, bk, wv, bv, wo)

    if "prog" not in _PROGRAM_CACHE:
        _PROGRAM_CACHE["prog"] = build_program()
    nc = _PROGRAM_CACHE["prog"]

    res = bass_utils.run_bass_kernel_spmd(
        nc, in_maps, core_ids=list(range(N_CORES)),
        trace=os.environ.get("KERNEL_TRACE", "") == "1",
        tmpdir=os.environ.get("KERNEL_TRACE_DIR") or None)
    _PROGRAM_CACHE["last_result"] = res

    out = np.zeros((B, S, D), dtype=np.float32)
    for c in range(N_CORES):
        out[c // QUADS] += np.asarray(res.results[c]["out"], dtype=np.float32)
    out += bo[None, None, :]
    return out


# revision 54
# speedup vs baseline: 1.0715x; 1.0129x over previous
"""Trainium2 Bass kernel for nn_CrossAttentionBlock (B=2, S=2048, D=1024, H=16, HD=64).

Sharding: 8 cores = 2 batches x 4 head-quads (4 heads each, E=256 channels).
Each core computes q/k/v projections for its quad, RoPE, SDPA, and a partial
output projection [S, D] (fp16); host sums the 4 partials per batch + bo.

Software-pipelined single pass per core. The scalar engine (exp, 128
[128,1024] tiles, ~1.1us each) and the tensor engine (~960 matmuls at
~0.4us issue cost each) are both near-saturated; everything else hides
under them:
  - startup is minimal: k-projection chunk 0, v s-tiles 0..3, q-projection
    of window 0. The rest of k/v projection enters window 0 as
    deadline-scheduled PE filler items, so the exp stream starts ~20us in.
  - 4 SDPA windows (512 q each), 32 units each (16 k-tiles x 2 head-pairs):
    scores pair (row-tiled) -> exp (double-buffered scores PSUM) -> ctx pair
    (col-tiled, PSUM accum). Denominators: exp tiles are pair-summed on DVE
    into fp16 (exact at these magnitudes), halving the ones-matmul count.
  - PSUM->SBUF casts run on GpSimd, rope swaps on the sync queue, output
    stores + normalize broadcast DMAs on GpSimd: DVE keeps only rope
    multiplies, pair-sums, normalize, reciprocal.
  - RoPE: even/odd permutation folded into w_q/w_k rows host-side ->
    rot-half rope out = q*cos + swap_halves(q)*sin_signed.
"""
import os
import sys

sys.path.insert(0, "/opt/trn_rl_repo")

import numpy as np
import ml_dtypes

BF16 = ml_dtypes.bfloat16

B, S, D, H = 2, 2048, 1024, 16
HD = D // H          # 64
DIM = HD // 2        # 32
QUADS = 4            # head groups of 4
E = D // QUADS       # 256 channels per core
ROPE_BASE = 10000.0
N_CORES = 8

KT = D // 128        # 8 k-tiles of the contraction dim
ST = S // 128        # 16 s-tiles
QC = S // 512        # 4 q-chunks (SDPA windows)


def _host_prep(x_q, x_kv, wq, bq, wk, bk, wv, bv, wo):
    """Per-core input maps, every tensor already in its SBUF layout."""
    perm = np.concatenate([np.arange(0, HD, 2), np.arange(1, HD, 2)])  # even|odd
    scale = 1.0 / np.sqrt(HD)

    freqs = np.exp(-np.arange(DIM, dtype=np.float64) * np.log(ROPE_BASE) / DIM)
    ang = np.arange(S, dtype=np.float64)[:, None] * freqs[None, :]     # [S, 32]
    cos = np.cos(ang).T                                                # [32, S]
    sin = np.sin(ang).T
    cos64 = np.concatenate([cos, cos], axis=0)                         # [64, S]
    sin64 = np.concatenate([-sin, sin], axis=0)
    cosT = np.concatenate([cos64, cos64], axis=0).astype(BF16)         # [128, S]
    sinT = np.concatenate([sin64, sin64], axis=0).astype(BF16)

    def x_layout(x):
        # x [S, D] -> xT [D, S] -> sc-major SBUF layout [128, 4*4096]:
        # chunk sc at cols sc*4096, inside: k-tile kt at +kt*512
        xT = x.T.reshape(KT, 128, QC, 512)
        return np.ascontiguousarray(
            xT.transpose(1, 2, 0, 3).reshape(128, KT * S)).astype(BF16)

    def w_layout(w, permute, s):
        # quad rows [256, 1024] (maybe permuted per head, scaled) -> d-major
        # [1024, 256] -> SBUF [128, 8*256] (k-tile kt at cols kt*256)
        blocks = []
        for h in range(4):
            wb = w[h * HD:(h + 1) * HD, :]
            if permute:
                wb = wb[perm, :]
            blocks.append(wb * s)
        wT = np.concatenate(blocks, axis=0).T                          # [1024, 256]
        return np.ascontiguousarray(
            wT.reshape(KT, 128, E).transpose(1, 0, 2).reshape(128, KT * E)
        ).astype(BF16)

    in_maps = []
    for c in range(N_CORES):
        b_ = c // QUADS
        g = c % QUADS
        hs = slice(g * E, (g + 1) * E)
        woT = wo[:, hs].T                                              # [256, 1024]
        wo_dev = np.ascontiguousarray(
            woT.reshape(2, 128, D).transpose(1, 0, 2).reshape(128, 2 * D)
        ).astype(BF16)
        in_maps.append({
            "xq": x_layout(x_q[b_]), "xkv": x_layout(x_kv[b_]),
            "wq": w_layout(wq[hs, :], True, scale),
            "wk": w_layout(wk[hs, :], True, 1.0),
            "wv": w_layout(wv[hs, :], False, 1.0),
            "wo": wo_dev,
            "cosT": np.ascontiguousarray(cosT),
            "sinT": np.ascontiguousarray(sinT),
            "ones_col": np.ones((128, 1), dtype=np.float16),
        })
    return in_maps


# ---------------------------------------------------------------------------
_PROGRAM_CACHE = {}


def _fixed_tile_context(tile_mod, bass_rust_mod, vector_clock_mod):
    """TileContext whose tail drain splits multi-sem waits into single-wait
    NOPs (this walrus rejects >1 sync-wait on one instruction)."""
    SyncInfo = bass_rust_mod.SyncInfo
    ScopedClock = vector_clock_mod.ScopedClock

    class TC(tile_mod.TileContext):
        def _drain_and_barrier(self, tick_clock, wait_clock):
            harvest = self.nc.sync.nop(nofuse=True)
            wait_clock.add_sem_waits(
                harvest.ins, ScopedClock({None: tick_clock.global_clock}))
            si = harvest.ins.sync_info
            waits = list(si.on_wait) if si is not None else []
            if len(waits) > 1:
                harvest.ins.sync_info = SyncInfo(
                    on_wait=[waits[0]], on_update=list(si.on_update))
                for w in waits[1:]:
                    nop = self.nc.sync.nop(nofuse=True)
                    nop.ins.sync_info = SyncInfo(on_wait=[w], on_update=[])
            self.nc.sync.drain()
            self.nc.all_engine_barrier()
            assert self.sems is not None
            popped = self.nc._tile_sem_poison_stack.pop()
            assert popped is self._sem_poison
            self.nc.clear_and_free_semaphores(list(self.sems.allocated().values()))
            self.nc.all_engine_barrier()

    return TC


def _split_multiwait_instructions(nc, mybir, SyncInfo):
    """This walrus build rejects >1 sync-wait per instruction; hoist extra
    waits onto single-wait NOPs inserted just before, on the same engine."""
    ctr = 0
    for blk in nc.m.functions[0].blocks:
        insts = blk.instructions
        i = 0
        while i < len(insts):
            inst = insts[i]
            si = inst.sync_info
            if si is not None and len(si.on_wait) > 1:
                waits = list(si.on_wait)
                inst.sync_info = SyncInfo(on_wait=[waits[-1]],
                                          on_update=list(si.on_update))
                nops = []
                for w in waits[:-1]:
                    nop = mybir.InstNoOp(name=f"waitsplit_{ctr}", ins=[], outs=[])
                    ctr += 1
                    nop.engine = inst.engine
                    nop.sync_info = SyncInfo(on_wait=[w], on_update=[])
                    nops.append(nop)
                insts[i:i] = nops
                i += len(nops)
            i += 1
    return ctr


def build_program(split_waits=True):
    import concourse.bass as bass
    import concourse.mybir as mybir
    import concourse.tile as tile
    import bass_rust
    from concourse import vector_clock

    f32 = mybir.dt.float32
    fp16 = mybir.dt.float16
    bf16 = mybir.dt.bfloat16
    Exp = mybir.ActivationFunctionType.Exp
    Ln = mybir.ActivationFunctionType.Ln
    mult = mybir.AluOpType.mult
    add = mybir.AluOpType.add

    gps_cast = os.environ.get("KERNEL_NO_GPS_CAST", "") != "1"

    nc = bass.Bass("TRN2", target_bir_lowering=False, debug=False,
                   num_devices=N_CORES)

    xq_d = nc.dram_tensor("xq", [128, KT * S], bf16, kind="ExternalInput").ap()
    xkv_d = nc.dram_tensor("xkv", [128, KT * S], bf16, kind="ExternalInput").ap()
    wq_d = nc.dram_tensor("wq", [128, KT * E], bf16, kind="ExternalInput").ap()
    wk_d = nc.dram_tensor("wk", [128, KT * E], bf16, kind="ExternalInput").ap()
    wv_d = nc.dram_tensor("wv", [128, KT * E], bf16, kind="ExternalInput").ap()
    wo_d = nc.dram_tensor("wo", [128, 2 * D], bf16, kind="ExternalInput").ap()
    cos_d = nc.dram_tensor("cosT", [128, S], bf16, kind="ExternalInput").ap()
    sin_d = nc.dram_tensor("sinT", [128, S], bf16, kind="ExternalInput").ap()
    ones_d = nc.dram_tensor("ones_col", [128, 1], fp16, kind="ExternalInput").ap()
    out = nc.dram_tensor("out", [S, D], fp16, kind="ExternalOutput").ap()

    TC = _fixed_tile_context(tile, bass_rust, vector_clock)

    with TC(nc) as tc:
        with tc.tile_pool(name="persist", bufs=1) as per, \
             tc.tile_pool(name="ps", bufs=1, space="PSUM") as psp, \
             tc.tile_pool(name="edram", bufs=1, space="DRAM") as edr:
            # ---- persistent SBUF ----
            xq_sb = per.tile([128, KT * S], bf16, tag="xq")
            xkv_sb = per.tile([128, KT * S], bf16, tag="xkv")
            wq_sb = per.tile([128, KT * E], bf16, tag="wq")
            wk_sb = per.tile([128, KT * E], bf16, tag="wk")
            wv_sb = per.tile([128, KT * E], bf16, tag="wv")
            wo_sb = per.tile([128, 2 * D], bf16, tag="wo")
            cos_sb = per.tile([128, S], bf16, tag="cos")
            sin_sb = per.tile([128, S], bf16, tag="sin")
            ones_sb = per.tile([128, 1], fp16, tag="ones")
            ones_bf = per.tile([128, 1], bf16, tag="onesb")
            qr_sb = [per.tile([128, S], bf16, tag=f"qr{p}", name=f"qr{p}")
                     for p in range(2)]
            kr_sb = [per.tile([128, S], bf16, tag=f"kr{p}", name=f"kr{p}")
                     for p in range(2)]
            v_sb = per.tile([128, ST * E], bf16, tag="v")
            ctxn_sb = [per.tile([128, S], bf16, tag=f"ctxn{p}", name=f"ctxn{p}")
                       for p in range(2)]
            rb_sb = per.tile([128, S], bf16, tag="rb")
            rsw_sb = per.tile([128, S], bf16, tag="rsw")
            rcos_sb = per.tile([128, S], bf16, tag="rcos")
            e_sb = [per.tile([128, 1024], bf16, tag=f"e{i}", name=f"e{i}")
                    for i in range(8)]
            et_sb = [per.tile([128, 1024], fp16, tag=f"et{p}", name=f"et{p}")
                     for p in range(2)]
            o_sb = [per.tile([128, D], fp16, tag=f"o{i}", name=f"o{i}")
                    for i in range(2)]
            linv_sb = per.tile([128, 512], f32, tag="linv")
            lbc_sb = [per.tile([128, 512], f32, tag=f"lbc{p}", name=f"lbc{p}")
                      for p in range(2)]
            lscr = edr.tile([4, 512], f32, tag="lscr")

            # ---- PSUM: 8 banks exactly ----
            s_ps = [psp.tile([128, 1024], f32, tag=f"s{i}", name=f"s{i}")
                    for i in range(2)]                       # banks 0-3
            ctx_ps = [psp.tile([128, 512], f32, tag=f"c{p}", name=f"c{p}")
                      for p in range(2)]                     # banks 4-5
            den_ps = psp.tile([128, 512], f32, tag="den")    # bank 6
            rot_ps = psp.tile([128, 512], f32, tag="rot")    # bank 7

            def cast(dst, src):
                nc.vector.tensor_copy(dst, src)

            rope_eng = (nc.gpsimd if os.environ.get("KERNEL_GPS_ROPE", "")
                        == "1" else nc.vector)

            def rope_chunk(dst, c0, ln):
                """dst[:, c0:c0+ln] = rb*cos + swap_halves(rb)*sin over the
                column range [c0, c0+ln) (positions match table columns)."""
                r = slice(c0, c0 + ln)
                for a, bdst in ((0, 32), (32, 0), (64, 96), (96, 64)):
                    nc.sync.dma_start(rsw_sb[bdst:bdst + 32, r],
                                      rb_sb[a:a + 32, r])
                rope_eng.tensor_tensor(rcos_sb[:, r], rb_sb[:, r],
                                       cos_sb[:, r], mult)
                rope_eng.tensor_tensor(rsw_sb[:, r], rsw_sb[:, r],
                                       sin_sb[:, r], mult)
                rope_eng.tensor_tensor(dst[:, r], rcos_sb[:, r],
                                       rsw_sb[:, r], add)

            def proj_mms(ps_tile, pc0, w_sb, x_sb, sc, pcol, n, kts):
                """Accumulate k-tiles kts of W^T[:, pcol:pcol+128] @ x chunk sc
                (n cols) into ps_tile[:, pc0:pc0+n]."""
                for kt in kts:
                    nc.tensor.matmul(
                        ps_tile[:, pc0:pc0 + n],
                        lhsT=w_sb[:, kt * E + pcol: kt * E + pcol + 128],
                        rhs=x_sb[:, sc * 4096 + kt * 512: sc * 4096 + kt * 512 + n],
                        start=(kt == 0), stop=(kt == KT - 1),
                        skip_group_check=True)

            def vproj(st, ps_tile, pc0):
                """v projection of s-tile st into ps_tile[:, pc0:pc0+E]."""
                sc, r = st // 4, (st % 4) * 128
                for kt in range(KT):
                    nc.tensor.matmul(
                        ps_tile[:, pc0:pc0 + E],
                        lhsT=xkv_sb[:, sc * 4096 + kt * 512 + r:
                                    sc * 4096 + kt * 512 + r + 128],
                        rhs=wv_sb[:, kt * E:(kt + 1) * E],
                        start=(kt == 0), stop=(kt == KT - 1),
                        skip_group_check=True)
                cast(v_sb[:, st * E:(st + 1) * E], ps_tile[:, pc0:pc0 + E])

            # ---- startup: staged loads interleaved with all projections ----
            # k chunks rotate through s_ps[0] halves, v tiles through s_ps[1]
            # halves, q chunk 0 uses the ctx banks. Emission order = PE order;
            # each stage's DMAs are emitted just before their consumers.
            nc.vector.memset(ones_bf[:, :], 1.0)

            # PE warm-up: the HAM clock gate holds the PE at 1.2 GHz until it
            # sees ~3.4us of sustained activity. The first projections are
            # gated on input DMAs (~12us), so without this the whole startup
            # runs at half clock. Dependency-free dummy matmuls on the free
            # rot bank fill the DMA lead-in and unlock 2.4 GHz; sized to end
            # before the first real matmul's inputs arrive.
            nc.vector.memset(rb_sb[0:1, 0:512], 0.0)
            for _ in range(16):
                nc.tensor.matmul(
                    rot_ps[0:1, 0:512], lhsT=ones_bf[0:1, 0:1],
                    rhs=rb_sb[0:1, 0:512], start=True, stop=True,
                    skip_group_check=True)

            def kp_chunk(sc, p):
                pc0 = (p % 2) * 512
                proj_mms(s_ps[0], pc0, wk_sb, xkv_sb, sc, p * 128, 512,
                         range(KT))
                c0 = sc * 512
                cast(rb_sb[:, c0:c0 + 512], s_ps[0][:, pc0:pc0 + 512])
                rope_chunk(kr_sb[p], c0, 512)

            nc.sync.dma_start(wk_sb[:, :], wk_d[:, :])
            nc.sync.dma_start(xkv_sb[:, 0:4096], xkv_d[:, 0:4096])
            nc.gpsimd.dma_start(cos_sb[:, :], cos_d[:, :])
            nc.gpsimd.dma_start(sin_sb[:, :], sin_d[:, :])
            nc.sync.dma_start(wv_sb[:, :], wv_d[:, :])
            nc.sync.dma_start(xkv_sb[:, 4096:8192], xkv_d[:, 4096:8192])
            kp_chunk(0, 0)
            kp_chunk(0, 1)
            nc.sync.dma_start(wq_sb[:, :], wq_d[:, :])
            nc.sync.dma_start(xq_sb[:, 0:4096], xq_d[:, 0:4096])
            nc.gpsimd.dma_start(wo_sb[:, :], wo_d[:, :])
            nc.gpsimd.dma_start(ones_sb[:, :], ones_d[:, :])
            for sc in range(1, QC):
                nc.gpsimd.dma_start(xq_sb[:, sc * 4096:(sc + 1) * 4096],
                                    xq_d[:, sc * 4096:(sc + 1) * 4096])
            for st in range(4):
                vproj(st, s_ps[1], (st % 2) * 512)
            kp_chunk(1, 0)
            kp_chunk(1, 1)
            for p in range(2):
                proj_mms(ctx_ps[p], 0, wq_sb, xq_sb, 0, p * 128, 512,
                         range(KT))
                cast(rb_sb[:, 0:512], ctx_ps[p][:, 0:512])
                rope_chunk(qr_sb[p], 0, 512)
            for st in range(4, 8):
                vproj(st, s_ps[1], (st % 2) * 512)
            for sc in range(2, QC):
                nc.sync.dma_start(xkv_sb[:, sc * 4096:(sc + 1) * 4096],
                                  xkv_d[:, sc * 4096:(sc + 1) * 4096])
                kp_chunk(sc, 0)
                kp_chunk(sc, 1)
                for st in range(4 * sc, 4 * sc + 4):
                    vproj(st, s_ps[1], (st % 2) * 512)

            # ---- window filler items -------------------------------------
            def qproj_items(qh):
                def it(p, qh=qh):
                    proj_mms(rot_ps, 0, wq_sb, xq_sb, qh, p * 128, 512,
                             range(KT))
                    c0 = qh * 512
                    cast(rb_sb[:, c0:c0 + 512], rot_ps[:, 0:512])
                    rope_chunk(qr_sb[p], c0, 512)
                return [lambda p=p: it(p) for p in range(2)]

            def oproj_items(qh, banks=None, act_cast=False):
                """Output projection of window qh's s-tiles. banks: list of
                (tile, col0) PSUM regions to rotate through (default rot).
                act_cast: alternate casts DVE/ACT (tail only, ACT idle)."""
                if banks is None:
                    banks = [(rot_ps, 0)]
                items = []
                for j, st in enumerate(range(qh * 4, qh * 4 + 4)):
                    for ch in range(2):
                        bt, bc = banks[(2 * j + ch) % len(banks)]
                        def it(st=st, ch=ch, bt=bt, bc=bc):
                            for p in range(2):
                                nc.tensor.matmul(
                                    bt[:, bc:bc + 512],
                                    lhsT=ctxn_sb[p][:, st * 128:(st + 1) * 128],
                                    rhs=wo_sb[:, p * D + ch * 512:
                                              p * D + ch * 512 + 512],
                                    start=(p == 0), stop=(p == 1),
                                    skip_group_check=True)
                            ob = o_sb[st % 2]
                            if act_cast and ch == 1:
                                nc.scalar.copy(ob[:, ch * 512:(ch + 1) * 512],
                                               bt[:, bc:bc + 512])
                            else:
                                cast(ob[:, ch * 512:(ch + 1) * 512],
                                     bt[:, bc:bc + 512])
                            if ch == 1:
                                nc.sync.dma_start(
                                    out[st * 128:(st + 1) * 128, :], ob[:, :])
                        items.append(it)
                return items

            # ---- SDPA windows ----
            pair_eng = (nc.gpsimd if os.environ.get("KERNEL_GPS_PAIR", "")
                        == "1" else nc.vector)
            uglob = 0
            carry = {}   # units of the next window emitted early (prologue)

            def emit_unit(qh, u):
                ki, p = u // 2, u % 2
                qs_ = slice(qh * 512, (qh + 1) * 512)
                sb = s_ps[u % 2]
                nc.tensor.matmul(
                    sb[:, 0:512],
                    lhsT=kr_sb[p][0:64, ki * 128:(ki + 1) * 128],
                    rhs=qr_sb[p][0:64, qs_],
                    tile_position=(0, 0), start=True, stop=True)
                nc.tensor.matmul(
                    sb[:, 512:1024],
                    lhsT=kr_sb[p][64:128, ki * 128:(ki + 1) * 128],
                    rhs=qr_sb[p][64:128, qs_],
                    tile_position=(64, 0), start=True, stop=True)
                e = e_sb[emit_unit.uglob % 8]
                emit_unit.uglob += 1
                nc.scalar.activation(e[:, :], sb[:, :], Exp)
                return e
            emit_unit.uglob = 0

            for qh in range(QC):
                qs = slice(qh * 512, (qh + 1) * 512)
                # (due_u, fn): emitted when u >= due_u, strict FIFO
                sched = []
                if qh > 0:
                    its = oproj_items(qh - 1)
                    for j, it in enumerate(its[:4]):
                        sched.append((2 + 3 * j, it))
                    for j, it in enumerate(its[4:]):
                        sched.append((18 + 3 * j, it))
                if qh < QC - 1:
                    for j, it in enumerate(qproj_items(qh + 1)):
                        sched.append((12 + 3 * j, it))
                sched.sort(key=lambda t: t[0])

                nc.vector.memset(den_ps[:, :], 1.0)

                def emit_ctx(u, e):
                    ki, p = u // 2, u % 2
                    for half in range(2):
                        nc.tensor.matmul(
                            ctx_ps[p][half * 64:(half + 1) * 64, :],
                            lhsT=v_sb[:, ki * E + (2 * p + half) * 64:
                                      ki * E + (2 * p + half) * 64 + 64],
                            rhs=e[:, half * 512:(half + 1) * 512],
                            tile_position=(0, half * 64),
                            start=(ki == 0), stop=(ki == ST - 1),
                            skip_group_check=True)

                def emit_den(u, direct=None):
                    # den matmuls for the pair completed at unit u (ki odd),
                    # deferred 2 slots so the pair-sum engine has slack.
                    # direct: list of e tiles to sum individually instead of
                    # via the pair tile (used for the final pair: shortens the
                    # window-end exp->TT->den->reciprocal critical chain)
                    ki, p = u // 2, u % 2
                    srcs = [et_sb[p]] if direct is None else direct
                    ones = ones_sb if direct is None else ones_bf
                    for j, src in enumerate(srcs):
                        for half in range(2):
                            g = 2 * p + half
                            nc.tensor.matmul(
                                den_ps[g * 32: g * 32 + 1, :],
                                lhsT=ones[:, :],
                                rhs=src[:, half * 512:(half + 1) * 512],
                                tile_position=(0, g * 32),
                                start=(ki == 1 and j == 0),
                                stop=(ki == ST - 1 and j == len(srcs) - 1),
                                skip_group_check=True)

                e_of_u = carry
                carry = {}
                si = 0
                for u in range(32):
                    ki, p = u // 2, u % 2
                    if u in e_of_u:
                        e = e_of_u[u]       # emitted in previous window's tail
                    else:
                        e = emit_unit(qh, u)
                        e_of_u[u] = e
                    if u >= 4 and ((u - 4) // 2) % 2 == 1:
                        # deferred den matmuls read et BEFORE this unit's
                        # pair-sum overwrites it
                        emit_den(u - 4)
                    if ki % 2 == 1:
                        # pair-sum exp tiles into fp16 (exact: values < 600)
                        pair_eng.tensor_tensor(et_sb[p][:, :],
                                               e_of_u[u - 2][:, :],
                                               e_of_u[u][:, :], add)
                    while si < len(sched) and sched[si][0] <= u:
                        sched[si][1]()
                        si += 1
                    # ctx lags 3 units so a window's first ctx (which waits on
                    # the previous normalize chain) doesn't stall the PE queue
                    if u >= 3:
                        emit_ctx(u - 3, e_of_u[u - 3])
                emit_ctx(29, e_of_u[29])
                emit_ctx(30, e_of_u[30])
                emit_ctx(31, e_of_u[31])
                emit_den(30)
                emit_den(31)
                while si < len(sched):
                    sched[si][1]()
                    si += 1

                # prologue: next window's first units keep the exp stream hot
                # while this window's normalize chain runs
                if qh < QC - 1:
                    for un in range(2):
                        carry[un] = emit_unit(qh + 1, un)

                # normalize: linv rows -> DRAM roundtrip broadcast -> ctx*linv
                nc.vector.reciprocal(linv_sb[:, :], den_ps[:, :])
                nc.sync.dma_start(lscr[:, :], linv_sb[0:128:32, :])
                for g, (p, half) in enumerate(((0, 0), (0, 1), (1, 0), (1, 1))):
                    nc.sync.dma_start(
                        lbc_sb[p][half * 64:(half + 1) * 64, :],
                        lscr[g:g + 1, :].partition_broadcast(64))
                for p in range(2):
                    nc.vector.tensor_tensor(
                        ctxn_sb[p][:, qs], ctx_ps[p][:, :], lbc_sb[p][:, :], mult)

            # tail: output projection of the last window on freed score banks
            for it in oproj_items(QC - 1, banks=[(s_ps[0], 0), (s_ps[0], 512),
                                                 (s_ps[1], 0), (s_ps[1], 512)],
                                  act_cast=True):
                it()

    if split_waits:
        _split_multiwait_instructions(nc, mybir, bass_rust.SyncInfo)
    return nc


def kernel(x_q, x_kv, wq, bq, wk, bk, wv, bv, wo, bo):
    from concourse import bass_utils

    x_q = np.asarray(x_q, dtype=np.float32)
    x_kv = np.asarray(x_kv, dtype=np.float32)
    wq = np.asarray(wq, dtype=np.float32); bq = np.asarray(bq, dtype=np.float32)
    wk = np.asarray(wk, dtype=np.float32); bk = np.asarray(bk, dtype=np.float32)
    wv = np.asarray(wv, dtype=np.float32); bv = np.asarray(bv, dtype=np.float32)
    wo = np.asarray(wo, dtype=np.float32); bo = np.asarray(bo, dtype=np.float32)

    assert not (np.any(bq) or np.any(bk) or np.any(bv)), \
        "nonzero qkv biases not supported by this build"

    in_maps = _host_prep(x_q, x_kv, wq, bq, wk, bk, wv, bv, wo)

    if "prog" not in _PROGRAM_CACHE:
        _PROGRAM_CACHE["prog"] = build_program()
    nc = _PROGRAM_CACHE["prog"]

    res = bass_utils.run_bass_kernel_spmd(
        nc, in_maps, core_ids=list(range(N_CORES)),
        trace=os.environ.get("KERNEL_TRACE", "") == "1",
        tmpdir=os.environ.get("KERNEL_TRACE_DIR") or None)
    _PROGRAM_CACHE["last_result"] = res

    out = np.zeros((B, S, D), dtype=np.float32)
    for c in range(N_CORES):
        out[c // QUADS] += np.asarray(res.results[c]["out"], dtype=np.float32)
    out += bo[None, None, :]
    return out
